# revision 13
# baseline (speedup 1.0000x reference)
"""Trainium2 Bass kernel for ContextQueryAttention (BiDAF-style trilinear attention).

Math (per batch b):
  S[n,m] = ctx[n]·w_c + q[m]·w_q + (ctx[n]*w_m)·q[m]
  A  = softmax_m(S + qmask_bias) ; Bm = softmax_n(S + cmask_bias)
  c2q = A @ q ;  q2c = A @ Bm^T @ ctx
  out = concat([ctx, c2q, ctx*c2q, ctx*q2c], -1)

Device strategy (per core, 4 batches, fp8 DoubleRow matmuls):
  T32 = 32·(ctx·wm)·q^T computed twice via fp8 DoubleRow (k=256/instr):
    - n-major S: + aug rows injecting 32·cwc[n] and c_mask log-bias, then
      Em = exp(S/32) in fp8 directly (B-path numerator, mask fused).
    - m-major ST: ET = exp(ST/32) fp8 (A-path; exp(cwc) cancels in A softmax).
  expqb[m] = exp(q·wq - 2 + qmask_bias)  (host-computed logits, device exp)
  B-path: C1raw = Em^T @ [ctx|1] ; C1s = fp8(0.25·expqb/colsum · C1raw)
  A-path: c2q_raw = ET^T @ (q·expqb) -> bf16 out ; rows = ET^T @ expqb
          q2c_raw = ET^T @ C1s -> fp8 out
  Host: divides by rows, upcasts, and assembles concat([ctx, c2q, ctx*c2q,
  ctx*q2c]) from shipped c2q_raw/q2c_raw/rows (ctx already on host).
  Batches are software-pipelined 2 deep (stage1 = loads+S/ST+exps,
  stage2 = attention matmuls+drains+stores) to avoid per-engine
  head-of-line blocking between dependent phases.

Sharding: batch data-parallel, 4 of 32 batches per NeuronCore, 8 cores.
"""

import numpy as np
import ml_dtypes

B, N, M, D = 32, 1024, 256, 512
NCORES = 8
BL = B // NCORES          # batches per core
NT = N // 128             # 8 context row tiles
MT = M // 128             # 2 query row tiles
DC = D // 128             # 4 feature chunks
SC = 32.0                 # wm pre-scale for fp8 conditioning (exp undoes it)
EB = -2.0                 # expqb bias keeping q·expqb in fp8 range
QSC = 0.25                # extra C1s scale keeping q2c_raw in fp8 range
CZB = -240.0              # aug czlog row value; ·150 then /32 => exp->0

F8NP = ml_dtypes.float8_e4m3
BFNP = ml_dtypes.bfloat16

_built = {}


def _build_nc(repeat=1):
    import concourse.bass as bass  # noqa: F401
    import concourse.mybir as mybir
    import concourse.tile as tile
    from concourse import bacc

    f32 = mybir.dt.float32
    f8 = mybir.dt.float8e4
    bf16 = mybir.dt.bfloat16
    EXP = mybir.ActivationFunctionType.Exp
    MUL = mybir.AluOpType.mult
    DR = mybir.MatmulPerfMode.DoubleRow

    nc = bacc.Bacc("TRN2", target_bir_lowering=False, debug=False)
    ctxT_d = nc.dram_tensor("ctxT8", (BL, 128, DC, N), f8, kind="ExternalInput")
    ctx_d = nc.dram_tensor("ctx8", (BL, 128, NT, 516), f8, kind="ExternalInput")
    qtw_d = nc.dram_tensor("qtw8", (BL, 128, DC, M), f8, kind="ExternalInput")
    qsb_d = nc.dram_tensor("qsb", (BL, 128, MT, 516), bf16, kind="ExternalInput")
    aug_d = nc.dram_tensor("aug", (BL, 2, 2, N), f8, kind="ExternalInput")
    augr_d = nc.dram_tensor("augr", (2, 2, M), f8, kind="ExternalInput")
    qb_d = nc.dram_tensor("qb", (128, BL, MT), f32, kind="ExternalInput")
    c2q_d = nc.dram_tensor("c2q", (BL, 128, NT, 512), bf16, kind="ExternalOutput")
    q2c_d = nc.dram_tensor("q2c", (BL, 128, NT, 512), f8, kind="ExternalOutput")
    rows_d = nc.dram_tensor("rows", (BL, 128, 32), f32, kind="ExternalOutput")

    ctxT_ap = ctxT_d.ap()
    ctx_ap = ctx_d.ap()
    qtw_ap = qtw_d.ap()
    qsb_ap = qsb_d.ap()
    aug_ap = aug_d.ap()
    c2q_ap = c2q_d.ap()
    q2c_ap = q2c_d.ap()
    rows_ap = rows_d.ap()

    with tile.TileContext(nc) as tc:
        with (
            tc.tile_pool(name="singles", bufs=1) as singles,
            tc.tile_pool(name="p_ctxT", bufs=3) as p_ctxT,
            tc.tile_pool(name="p_ctx", bufs=3) as p_ctx,
            tc.tile_pool(name="p_qtw", bufs=3) as p_qtw,
            tc.tile_pool(name="p_q", bufs=3) as p_q,
            tc.tile_pool(name="p_aug", bufs=3) as p_aug,
            tc.tile_pool(name="p_qs", bufs=2) as p_qs,
            tc.tile_pool(name="p_em", bufs=2) as p_em,
            tc.tile_pool(name="p_et", bufs=2) as p_et,
            tc.tile_pool(name="p_c1s", bufs=2) as p_c1s,
            tc.tile_pool(name="p_small", bufs=2) as p_small,
            tc.tile_pool(name="p_oc", bufs=2) as p_oc,
            tc.tile_pool(name="p_oq", bufs=2) as p_oq,
            tc.tile_pool(name="p_orow", bufs=2) as p_orow,
            tc.tile_pool(name="ps_s", bufs=3, space="PSUM") as ps_s,
            tc.tile_pool(name="ps_cq", bufs=2, space="PSUM") as ps_cq,
            tc.tile_pool(name="ps_tiny", bufs=1, space="PSUM") as ps_tiny,
        ):
            augr_sb = singles.tile([2, 2, M], f8)
            nc.sync.dma_start(augr_sb, augr_d.ap())
            qb_sb = singles.tile([128, BL, MT], f32)
            nc.sync.dma_start(qb_sb, qb_d.ap())

            def s1a(b):
                """Loads, expqb, qs for batch b."""
                st = {"b": b}
                ctxT = p_ctxT.tile([128, DC, N], f8, tag="ctxT")
                nc.sync.dma_start(ctxT, ctxT_ap[b])
                qtw = p_qtw.tile([128, DC, M], f8, tag="qtw")
                nc.sync.dma_start(qtw, qtw_ap[b])
                augl = p_aug.tile([2, 2, N], f8, tag="aug")
                nc.sync.dma_start(augl, aug_ap[b])
                qsb = p_q.tile([128, MT, 516], bf16, tag="qsb")
                nc.sync.dma_start(qsb, qsb_ap[b])
                ctx = p_ctx.tile([128, NT, 516], f8, tag="ctx")
                nc.sync.dma_start(ctx, ctx_ap[b])

                expqb = p_small.tile([128, MT], f32, tag="expqb")
                nc.scalar.activation(expqb, qb_sb[:, b, :], EXP, scale=1.0)
                qs = p_qs.tile([128, MT, 516], f8, tag="qs")
                for mt in range(MT):
                    nc.gpsimd.tensor_scalar(
                        qs[:, mt, :], qsb[:, mt, :], expqb[:, mt:mt + 1], None, MUL,
                    )

                st.update(ctx=ctx, ctxT=ctxT, qtw=qtw, augl=augl,
                          qs=qs, expqb=expqb)
                return st

            def s1b(st):
                """S (n-major) in nt-pairs; Em = exp(S/32) fp8, mask+cwc fused."""
                ctxT, qtw, augl = st["ctxT"], st["qtw"], st["augl"]
                Em = p_em.tile([128, NT, M], f8, tag="Em")
                for g in range(NT // 2):
                    s_ps = ps_s.tile([128, 2, 256], f32, tag="ps")
                    for t in range(2):
                        nt = 2 * g + t
                        nchunk = slice(nt * 128, (nt + 1) * 128)
                        for dp in range(2):
                            nc.tensor.matmul(
                                s_ps[:, t, 0:M],
                                ctxT[:, 2 * dp:2 * dp + 2, nchunk],
                                qtw[:, 2 * dp:2 * dp + 2, :],
                                start=(dp == 0), stop=False, perf_mode=DR,
                            )
                        nc.tensor.matmul(
                            s_ps[:, t, 0:M],
                            augl[:, :, nchunk],
                            augr_sb,
                            start=False, stop=True, perf_mode=DR,
                        )
                    nc.scalar.activation(
                        Em[:, 2 * g:2 * g + 2, :], s_ps, EXP, scale=1.0 / SC,
                    )

                st["Em"] = Em

            def s1c(st):
                """ST (m-major); ET = exp(ST/32) fp8 (no bias needed)."""
                ctxT, qtw = st["ctxT"], st["qtw"]
                ET = p_et.tile([128, MT, N], f8, tag="ET")
                for mc in range(MT):
                    mchunk = slice(mc * 128, (mc + 1) * 128)
                    for half in range(2):
                        st_ps = ps_s.tile([128, 2, 256], f32, tag="ps")
                        for pc2 in range(2):
                            npiece = slice(half * 512 + pc2 * 256,
                                           half * 512 + pc2 * 256 + 256)
                            for dp in range(2):
                                nc.tensor.matmul(
                                    st_ps[:, pc2, :],
                                    qtw[:, 2 * dp:2 * dp + 2, mchunk],
                                    ctxT[:, 2 * dp:2 * dp + 2, npiece],
                                    start=(dp == 0), stop=(dp == 1), perf_mode=DR,
                                )
                        nc.scalar.activation(
                            ET[:, mc, half * 512:half * 512 + 512], st_ps, EXP,
                            scale=1.0 / SC,
                        )

                st["ET"] = ET

            def s2a(st):
                """c2q_raw = ET^T @ qs ; rows = ET^T @ expqb ; drains+stores."""
                b, qs, ET = st["b"], st["qs"], st["ET"]
                rows_cs = ps_tiny.tile([128, 512], f32, tag="rows")
                st["rows_cs"] = rows_cs
                c2q_sb = p_oc.tile([128, NT, 512], bf16, tag="c2q_sb")
                for g in range(NT // 2):
                    c_ps = ps_cq.tile([128, 2, 512], f32, tag="ps")
                    for t in range(2):
                        nt = 2 * g + t
                        nchunk = slice(nt * 128, (nt + 1) * 128)
                        for pc in range(2):
                            nc.tensor.matmul(
                                c_ps[:, t, pc * 256:(pc + 1) * 256],
                                ET[:, :, nchunk],
                                qs[:, :, pc * 256:(pc + 1) * 256],
                                start=True, stop=True, perf_mode=DR,
                            )
                        nc.tensor.matmul(
                            rows_cs[:, nt * 2:nt * 2 + 2],
                            ET[:, :, nchunk],
                            qs[:, :, 512:514],
                            start=True, stop=True, perf_mode=DR,
                        )
                    if g < 3:
                        nc.scalar.copy(c2q_sb[:, 2 * g:2 * g + 2, :], c_ps)
                    else:
                        nc.vector.tensor_copy(c2q_sb[:, 2 * g:2 * g + 2, :], c_ps)
                    if g == 1:
                        nc.scalar.dma_start(c2q_ap[b, :, 0:4, :], c2q_sb[:, 0:4, :])
                    elif g == 3:
                        nc.scalar.dma_start(c2q_ap[b, :, 4:8, :], c2q_sb[:, 4:8, :])

            def s2b(st):
                """C1raw = Em^T @ [ctx|1] ; C1s = fp8(QSC·expqb/colsum·C1raw)."""
                ctx, expqb, Em, rows_cs = (
                    st["ctx"], st["expqb"], st["Em"], st["rows_cs"])
                C1s = p_c1s.tile([128, MT, 512], f8, tag="C1s")
                rrt = p_small.tile([128, MT], f32, tag="rrt")
                rct = p_small.tile([128, 4], f32, tag="rct")
                c1t = ps_cq.tile([128, 2, 512], f32, tag="ps")
                for mc in range(MT):
                    c1_ps = c1t[:, mc, :]
                    mchunk = slice(mc * 128, (mc + 1) * 128)
                    for pc in range(2):
                        for kp in range(4):
                            ksl = slice(2 * kp, 2 * kp + 2)
                            nc.tensor.matmul(
                                c1_ps[:, pc * 256:(pc + 1) * 256],
                                Em[:, ksl, mchunk],
                                ctx[:, ksl, pc * 256:(pc + 1) * 256],
                                start=(kp == 0), stop=(kp == 3), perf_mode=DR,
                            )
                    for kp in range(4):
                        ksl = slice(2 * kp, 2 * kp + 2)
                        nc.tensor.matmul(
                            rows_cs[:, 16 + mc * 2:16 + mc * 2 + 2],
                            Em[:, ksl, mchunk],
                            ctx[:, ksl, 512:514],
                            start=(kp == 0), stop=(kp == 3), perf_mode=DR,
                        )
                    nc.vector.reciprocal(
                        rct[:, mc * 2:mc * 2 + 2],
                        rows_cs[:, 16 + mc * 2:16 + mc * 2 + 2],
                    )
                    nc.vector.tensor_scalar(
                        rrt[:, mc:mc + 1], expqb[:, mc:mc + 1], QSC, None, MUL,
                    )
                    nc.vector.tensor_tensor(
                        rrt[:, mc:mc + 1], rrt[:, mc:mc + 1],
                        rct[:, mc * 2:mc * 2 + 1], MUL,
                    )
                    nc.vector.tensor_scalar(
                        C1s[:, mc, :], c1_ps, rrt[:, mc:mc + 1], None, MUL,
                    )

                st["C1s"] = C1s

            def s2c(st):
                """q2c_raw = ET^T @ C1s (fp8 out); stores."""
                b, ET, C1s, rows_cs = st["b"], st["ET"], st["C1s"], st["rows_cs"]
                q2c_sb = p_oq.tile([128, NT, 512], f8, tag="q2c_sb")
                for g in range(NT // 2):
                    q_ps = ps_cq.tile([128, 2, 512], f32, tag="ps")
                    for t in range(2):
                        nt = 2 * g + t
                        nchunk = slice(nt * 128, (nt + 1) * 128)
                        for pc in range(2):
                            nc.tensor.matmul(
                                q_ps[:, t, pc * 256:(pc + 1) * 256],
                                ET[:, :, nchunk],
                                C1s[:, :, pc * 256:(pc + 1) * 256],
                                start=True, stop=True, perf_mode=DR,
                            )
                    nc.vector.tensor_copy(q2c_sb[:, 2 * g:2 * g + 2, :], q_ps)
                    if g == 1:
                        nc.gpsimd.dma_start(q2c_ap[b, :, 0:4, :], q2c_sb[:, 0:4, :])
                    elif g == 3:
                        nc.gpsimd.dma_start(q2c_ap[b, :, 4:8, :], q2c_sb[:, 4:8, :])

                rows_sb = p_orow.tile([128, 32], f32, tag="rows_sb")
                nc.vector.tensor_copy(rows_sb, rows_cs[:, 0:32])
                nc.scalar.dma_start(rows_ap[b], rows_sb)

            # 2-deep software pipeline over batches
            n_iters = repeat * BL
            prev = s1a(0)
            s1b(prev)
            s1c(prev)
            for it in range(1, n_iters):
                cur = s1a(it % BL)
                s1b(cur)
                s1c(cur)
                s2a(prev)
                s2b(prev)
                s2c(prev)
                prev = cur
            s2a(prev)
            s2b(prev)
            s2c(prev)

    nc.compile()
    return nc


def get_nc(repeat=1):
    key = ("nc", repeat)
    if key not in _built:
        _built[key] = _build_nc(repeat)
    return _built[key]


def _host_prep(context, query, c_mask, q_mask, w):
    context = np.ascontiguousarray(np.asarray(context, dtype=np.float32))
    query = np.ascontiguousarray(np.asarray(query, dtype=np.float32))
    c_mask = np.asarray(c_mask)
    q_mask = np.asarray(q_mask)
    w = np.asarray(w, dtype=np.float32).reshape(3 * D)
    wq, wc, wm = w[0:D], w[D:2 * D], w[2 * D:3 * D]

    augr = np.zeros((2, 2, M), dtype=F8NP)
    augr[0, 0, :] = 1.0
    augr[1, 0, :] = 150.0

    in_maps = []
    for c in range(NCORES):
        bs = slice(c * BL, (c + 1) * BL)
        ctx = context[bs]                     # [BL, N, D]
        qry = query[bs]                       # [BL, M, D]
        cm = c_mask[bs]
        qm = q_mask[bs]

        ctxT8 = np.ascontiguousarray(
            ctx.reshape(BL, N, DC, 128).transpose(0, 3, 2, 1)).astype(F8NP)
        ctx8 = np.empty((BL, 128, NT, 516), dtype=F8NP)
        ctx8[:, :, :, 0:512] = ctx.reshape(BL, NT, 128, D).transpose(0, 2, 1, 3)
        ctx8[:, :, :, 512:516] = 1.0
        qtw8 = np.ascontiguousarray(
            (qry * (wm * SC)).reshape(BL, M, DC, 128).transpose(0, 3, 2, 1)
        ).astype(F8NP)
        qsb = np.empty((BL, 128, MT, 516), dtype=BFNP)
        qsb[:, :, :, 0:512] = qry.reshape(BL, MT, 128, D).transpose(0, 2, 1, 3)
        qsb[:, :, :, 512:516] = 1.0

        aug = np.zeros((BL, 2, 2, N), dtype=F8NP)
        aug[:, 0, 0, :] = (SC * (ctx @ wc)).astype(F8NP)
        aug[:, 1, 0, :] = np.where(cm, 0.0, CZB).astype(F8NP)

        qwq = qry @ wq                        # [BL, M]
        qb = (qwq + np.where(qm, EB, -30000.0)).astype(np.float32)
        qb = np.ascontiguousarray(qb.reshape(BL, MT, 128).transpose(2, 0, 1))

        in_maps.append({
            "ctxT8": ctxT8,
            "ctx8": ctx8,
            "qtw8": qtw8,
            "qsb": qsb,
            "aug": aug,
            "augr": augr,
            "qb": qb,
        })
    return in_maps


def run_on_device(in_maps, trace=False, repeat=1, **kw):
    from concourse.bass_utils import run_bass_kernel_spmd

    nc = get_nc(repeat)
    return run_bass_kernel_spmd(
        nc, in_maps, core_ids=list(range(NCORES)), trace=trace, **kw
    )


def _assemble(context, results):
    context = np.asarray(context, dtype=np.float32)
    out = np.empty((B, N, 4 * D), dtype=np.float32)
    for c, r in enumerate(results):
        bs = slice(c * BL, (c + 1) * BL)
        ctx = context[bs]
        c2q_raw = np.asarray(r["c2q"]).astype(np.float32)   # [BL,128,NT,512]
        c2q_raw = c2q_raw.transpose(0, 2, 1, 3).reshape(BL, N, D)
        q2c_raw = np.asarray(r["q2c"]).astype(np.float32)
        q2c_raw = q2c_raw.transpose(0, 2, 1, 3).reshape(BL, N, D)
        rows = np.asarray(r["rows"])[:, :, 0:16].reshape(BL, 128, NT, 2)[:, :, :, 0]
        rows = rows.transpose(0, 2, 1).reshape(BL, N)
        inv = 1.0 / rows[:, :, None]
        c2q = c2q_raw * inv
        q2c = q2c_raw * (inv / QSC)
        o = out[bs]
        o[:, :, 0:D] = ctx
        o[:, :, D:2 * D] = c2q
        o[:, :, 2 * D:3 * D] = ctx * c2q
        o[:, :, 3 * D:4 * D] = ctx * q2c
    return out


def kernel(context, query, c_mask, q_mask, w):
    in_maps = _host_prep(context, query, c_mask, q_mask, w)
    res = run_on_device(in_maps)
    return _assemble(context, res.results)
